# revision 1
# baseline (speedup 1.0000x reference)
"""Trainium2 Bass kernel for ContrastiveVideoAudioSimilarity.

Math (per batch element b, fully folded form):
  q        = probe @ wq.T + bq                      # [1024] -> heads [16, 64]
  ck[h,:]  = q[h] @ wk[h*64:(h+1)*64, :]            # [16, 1024]  (host folded)
  scores   = x @ ck.T / 8                           # [T*S, 16]; bk shift cancels in softmax
  attn     = softmax over S
  cx[t,h]  = sum_s attn[s,h] * x[t,s,:]             # [T, 16, 1024]
  ctx[t,h*64+d] = cx[t,h] @ wv[h*64+d,:] + bv       # per-head V proj of pooled vector
  pooled   = ctx @ wo.T + bo ; LayerNorm ; proj ; L2-normalize both sides; scaled dot.

Sharding: batch B=8, one batch element per NeuronCore (8 cores), params replicated.
Host precomputes folded/transposed weights (CKT, wvT, woT, projT) so the device
never touches wq/wk and needs no on-device weight transposes.
"""

import sys

for _p in ("/opt/trn_rl_repo", "/root/.axon_site/_ro/trn_rl_repo"):
    if _p not in sys.path:
        sys.path.insert(0, _p)

import numpy as np
import ml_dtypes

import concourse.bass as bass
import concourse.tile as tile
from concourse import bacc, mybir
from concourse.masks import make_identity

F32 = mybir.dt.float32
F32R = mybir.dt.float32r
BF16 = mybir.dt.bfloat16

B, T, S, DV, DA, NH, DH, L2 = 8, 32, 256, 1024, 512, 16, 64, 2048
EPS = 1e-6
FPG = 4  # frames per group


def build_nc(n_groups=T // FPG):
    """Build the per-core Bass program. n_groups*FPG = number of frames processed."""
    nT = n_groups * FPG  # frames
    nc = bacc.Bacc("TRN2", target_bir_lowering=False, debug=False)

    xv = nc.dram_tensor("xv", [nT * S, DV], F32, kind="ExternalInput").ap()
    aud = nc.dram_tensor("aud", [L2, DA], F32, kind="ExternalInput").ap()
    cktb = nc.dram_tensor("cktb", [128, 128], BF16, kind="ExternalInput").ap()
    wvt = nc.dram_tensor("wvt", [128, 8, DV], BF16, kind="ExternalInput").ap()
    bvp = nc.dram_tensor("bvp", [128, 8], F32, kind="ExternalInput").ap()
    wot = nc.dram_tensor("wot", [128, 8, DV], BF16, kind="ExternalInput").ap()
    bo2 = nc.dram_tensor("bo2", [1, DV], F32, kind="ExternalInput").ap()
    lng = nc.dram_tensor("lng", [1, DV], F32, kind="ExternalInput").ap()
    lnb = nc.dram_tensor("lnb", [1, DV], F32, kind="ExternalInput").ap()
    pjt = nc.dram_tensor("pjt", [128, 8, DA], BF16, kind="ExternalInput").ap()
    pjb = nc.dram_tensor("pjb", [1, DA], F32, kind="ExternalInput").ap()
    sca = nc.dram_tensor("sca", [1, 2], F32, kind="ExternalInput").ap()
    out = nc.dram_tensor("out", [nT, L2], F32, kind="ExternalOutput").ap()

    with tile.TileContext(nc) as tc:
        with (
            tc.tile_pool(name="const", bufs=1) as constp,
            tc.tile_pool(name="persist", bufs=1) as persist,
            tc.tile_pool(name="dram", bufs=1, space="DRAM") as dramp,
        ):
            # ---- constants / params resident in SBUF ----
            idb = constp.tile([128, 128], BF16)
            make_identity(nc, idb)
            idf = constp.tile([128, 128], F32)
            make_identity(nc, idf)
            ck_s = constp.tile([128, 128], BF16)
            nc.gpsimd.dma_start(ck_s[:], cktb)
            ck_v = ck_s.rearrange("p (co h) -> p co h", co=8)
            wvt_s = persist.tile([128, 8, DV], BF16)
            nc.gpsimd.dma_start(wvt_s[:], wvt)
            bvp_s = constp.tile([128, 8], F32)
            nc.gpsimd.dma_start(bvp_s[:], bvp)
            wot_s = persist.tile([128, 8, DV], BF16)
            nc.gpsimd.dma_start(wot_s[:], wot)
            pjt_s = persist.tile([128, 8, DA], BF16)
            nc.gpsimd.dma_start(pjt_s[:], pjt)
            def bcast_load(src, parts, free):
                """Materialize a [1, free] DRAM row broadcast across `parts` partitions."""
                t_ = constp.tile([parts, free], F32)
                src_b = bass.AP(
                    tensor=src.tensor, offset=src.offset,
                    ap=[[0, parts]] + list(src.ap[1:]),
                )
                nc.gpsimd.dma_start(out=t_[:], in_=src_b)
                return t_

            bo2_s = bcast_load(bo2, nT, DV)
            lng_s = bcast_load(lng, nT, DV)
            lnb_s = bcast_load(lnb, nT, DV)
            pjb_s = bcast_load(pjb, nT, DA)
            sca_s = bcast_load(sca, nT, 2)
            epsb = constp.tile([128, 1], F32)
            nc.vector.memset(epsb, EPS)

            # cx^T accumulator: cxt[ci, co, t, h] = cx[t, h, co*128+ci]
            cxt = persist.tile([128, 8, nT, NH], BF16)

            # ---- audio prep (independent; scheduler runs it early) ----
            audtb = persist.tile([128, 16, 4, 128], BF16)  # [d_in, lblk, d_out, l_in]
            rl_b = constp.tile([T, L2], F32)  # 1/||audio_l|| bcast on partitions
            with tc.tile_pool(name="audp", bufs=1) as audp:
                audb = audp.tile([128, 16, DA], BF16)  # [l_in, l_blk, d]
                nc.gpsimd.dma_start(
                    out=audb[:], in_=aud.rearrange("(b a) d -> a b d", a=128)
                )
                ast = audp.tile([128, 16, 6], F32)
                amv = audp.tile([128, 16, 2], F32)
                for b in range(16):
                    nc.sync.dma_start_transpose(audtb[:, b, :, :], audb[:, b, :])
                    nc.vector.bn_stats(out=ast[:, b, :], in_=audb[:, b, :])
                    nc.vector.bn_aggr(out=amv[:, b, :], in_=ast[:, b, :])
                ssum = audp.tile([128, 16], F32)
                nc.vector.tensor_tensor(
                    out=ssum[:], in0=amv[:, :, 0], in1=amv[:, :, 0],
                    op=mybir.AluOpType.mult,
                )
                nc.vector.tensor_add(ssum[:], ssum[:], amv[:, :, 1])
                # ||a_l|| = sqrt(DA * (var + mu^2))
                nc.scalar.activation(
                    out=ssum[:], in_=ssum[:],
                    func=mybir.ActivationFunctionType.Sqrt, scale=float(DA),
                )
                rml = audp.tile([128, 16], F32)
                nc.vector.reciprocal(out=rml[:], in_=ssum[:])
                # flat l-order roundtrip via DRAM: rl_flat[b*128+a] = rml[a, b]
                rld = dramp.tile([L2], F32)
                nc.scalar.dma_start(
                    out=rld.rearrange("(b a) -> a b", a=128), in_=rml[:]
                )
                nc.gpsimd.dma_start(
                    out=rl_b[:nT],
                    in_=bass.AP(tensor=rld.tensor, offset=rld.offset,
                                ap=[[0, nT]] + list(rld.ap)),
                )

            # ---- stage 1: per-frame attention pooling in x-space ----
            with (
                tc.tile_pool(name="xb", bufs=2) as xbp,
                tc.tile_pool(name="xt", bufs=2) as xtp,
                tc.tile_pool(name="sm", bufs=2) as smp_,
                tc.tile_pool(name="at", bufs=2) as atp_,
                tc.tile_pool(name="ps_sc", bufs=3, space="PSUM") as ps_sc,
                tc.tile_pool(name="ps_at", bufs=2, space="PSUM") as ps_at,
                tc.tile_pool(name="ps_cx", bufs=3, space="PSUM") as ps_cx,
            ):
                for g in range(n_groups):
                    # load FPG frames, cast f32 -> bf16 in DMA
                    xbg = xbp.tile([128, FPG, 2, DV], BF16)  # [si, f, so, c]
                    nc.gpsimd.dma_start(
                        out=xbg[:],
                        in_=xv[g * FPG * S:(g + 1) * FPG * S, :].rearrange(
                            "(f so si) c -> si f so c", f=FPG, so=2
                        ),
                    )
                    # transpose to [ci, f, so, co, si]
                    xtg = xtp.tile([128, FPG, 2, 8, 128], BF16)
                    for f in range(FPG):
                        for so in range(2):
                            nc.sync.dma_start_transpose(
                                xtg[:, f, so, :, :], xbg[:, f, so, :]
                            )
                    # scores^T: [16@32f, s=256] per frame, packed on partitions
                    scp = ps_sc.tile([128, S], F32)
                    for f in range(FPG):
                        for co in range(8):
                            nc.tensor.matmul(
                                scp[32 * f:32 * f + 16, :],
                                ck_v[:, co, :],
                                xtg[:, f, :, co, :],
                                start=(co == 0),
                                stop=(co == 7),
                                tile_position=(0, 32 * f),
                            )
                    # softmax over s (free dim), per written frame region
                    negm = smp_.tile([128, 1], F32)
                    et = smp_.tile([128, S], F32)
                    esum = smp_.tile([128, 1], F32)
                    rsum = smp_.tile([128, 1], F32)
                    attn_t = atp_.tile([128, S], BF16)  # attn^T [h@32f, s]
                    for f in range(FPG):
                        r = slice(32 * f, 32 * f + 16)
                        nc.vector.reduce_max(
                            out=negm[r], in_=scp[r], axis=mybir.AxisListType.X,
                            negate=True,
                        )
                        nc.scalar.activation(
                            out=et[r], in_=scp[r],
                            func=mybir.ActivationFunctionType.Exp,
                            bias=negm[r], scale=1.0, accum_out=esum[r],
                        )
                        nc.vector.reciprocal(out=rsum[r], in_=esum[r])
                        nc.vector.tensor_scalar_mul(attn_t[r], et[r], rsum[r])
                    # transpose attn to [s, h] per frame half
                    attn_s = atp_.tile([128, FPG, 2, NH], BF16)
                    for f in range(FPG):
                        for so in range(2):
                            atp = ps_at.tile([128, NH], BF16)
                            nc.tensor.transpose(
                                atp[:],
                                attn_t[32 * f:32 * f + 16, 128 * so:128 * (so + 1)],
                                idb[32 * f:32 * f + 16, 32 * f:32 * f + 16],
                                tile_position=(32 * f, 0),
                            )
                            nc.vector.tensor_copy(out=attn_s[:, f, so, :], in_=atp[:])
                    # cx^T[c, h] = sum_s xb[s, c] * attn[s, h]
                    for f in range(FPG):
                        cxp = ps_cx.tile([128, 128], F32)
                        for co in range(8):
                            for so in range(2):
                                nc.tensor.matmul(
                                    cxp[:, co * NH:(co + 1) * NH],
                                    xbg[:, f, so, co * 128:(co + 1) * 128],
                                    attn_s[:, f, so, :],
                                    start=(so == 0),
                                    stop=(so == 1),
                                )
                        t = g * FPG + f
                        nc.vector.tensor_copy(
                            out=cxt[:, :, t, :],
                            in_=cxp.rearrange("p (co h) -> p co h", co=8),
                        )

            # ---- stage 2: per-head V-projection  ctx^T[e, t] ----
            ctxt = persist.tile([128, 8, nT], BF16)
            with tc.tile_pool(name="ps2", bufs=2, space="PSUM") as ps2:
                for eo in range(8):
                    ctp = ps2.tile([128, nT], F32)
                    for hh in range(2):
                        h = 2 * eo + hh
                        for co in range(8):
                            nc.tensor.matmul(
                                ctp[64 * hh:64 * (hh + 1), :],
                                wvt_s[:, co, h * DH:(h + 1) * DH],
                                cxt[:, co, :, h],
                                start=(co == 0),
                                stop=(co == 7),
                            )
                    # += bv (attn sums to 1), copy out
                    nc.vector.tensor_scalar_add(
                        out=ctxt[:, eo, :], in0=ctp[:], scalar1=bvp_s[:, eo:eo + 1]
                    )

            # ---- stage 3: wo projection + LayerNorm ----
            with (
                tc.tile_pool(name="s3", bufs=1) as s3,
                tc.tile_pool(name="ps3", bufs=2, space="PSUM") as ps3,
                tc.tile_pool(name="ps3t", bufs=2, space="PSUM") as ps3t,
            ):
                pooled = s3.tile([nT, DV], F32)
                for n in range(2):
                    pp = ps3.tile([nT, 512], F32, tag="pp")
                    for eo in range(8):
                        nc.tensor.matmul(
                            pp[:],
                            ctxt[:, eo, :],
                            wot_s[:, eo, 512 * n:512 * (n + 1)],
                            start=(eo == 0),
                            stop=(eo == 7),
                        )
                    nc.vector.tensor_tensor(
                        out=pooled[:, 512 * n:512 * (n + 1)], in0=pp[:],
                        in1=bo2_s[:, 512 * n:512 * (n + 1)],
                        op=mybir.AluOpType.add,
                    )
                # LayerNorm over DV
                lst = s3.tile([nT, 2, 6], F32)
                nc.vector.bn_stats(out=lst[:, 0, :], in_=pooled[:, 0:512])
                nc.vector.bn_stats(out=lst[:, 1, :], in_=pooled[:, 512:1024])
                lmv = s3.tile([nT, 2], F32)
                nc.vector.bn_aggr(out=lmv[:], in_=lst[:])
                sd = s3.tile([nT, 1], F32)
                nc.scalar.activation(
                    out=sd[:], in_=lmv[:, 1:2],
                    func=mybir.ActivationFunctionType.Sqrt, bias=epsb[:nT],
                )
                rstd = s3.tile([nT, 1], F32)
                nc.vector.reciprocal(out=rstd[:], in_=sd[:])
                nc.vector.tensor_scalar(
                    out=pooled[:], in0=pooled[:],
                    scalar1=lmv[:, 0:1], scalar2=rstd[:],
                    op0=mybir.AluOpType.subtract, op1=mybir.AluOpType.mult,
                )
                nc.vector.tensor_tensor(
                    out=pooled[:], in0=pooled[:],
                    in1=lng_s[:], op=mybir.AluOpType.mult,
                )
                nc.vector.tensor_tensor(
                    out=pooled[:], in0=pooled[:],
                    in1=lnb_s[:], op=mybir.AluOpType.add,
                )
                # transpose pooled -> [f, t]
                plt = s3.tile([128, 8, nT], BF16)
                for fo in range(8):
                    ptp = ps3t.tile([128, nT], F32, tag="ptp")
                    nc.tensor.transpose(
                        ptp[:], pooled[:, 128 * fo:128 * (fo + 1)], idf[:nT, :nT]
                    )
                    nc.vector.tensor_copy(out=plt[:, fo, :], in_=ptp[:])
                # audio-dim projection
                vtp = ps3.tile([nT, DA], F32, tag="pp")
                for fo in range(8):
                    nc.tensor.matmul(
                        vtp[:],
                        plt[:, fo, :],
                        pjt_s[:, fo, :],
                        start=(fo == 0),
                        stop=(fo == 7),
                    )
                vt = s3.tile([nT, DA], F32)
                nc.vector.tensor_tensor(
                    out=vt[:], in0=vtp[:], in1=pjb_s[:],
                    op=mybir.AluOpType.add,
                )
                # s_t = exp(logit_scale) / ||vt||
                vst = s3.tile([nT, 6], F32)
                nc.vector.bn_stats(out=vst[:], in_=vt[:])
                vmv = s3.tile([nT, 2], F32)
                nc.vector.bn_aggr(out=vmv[:], in_=vst[:])
                vss = s3.tile([nT, 1], F32)
                nc.vector.tensor_tensor(
                    out=vss[:], in0=vmv[:, 0:1], in1=vmv[:, 0:1],
                    op=mybir.AluOpType.mult,
                )
                nc.vector.tensor_add(vss[:], vss[:], vmv[:, 1:2])
                nc.scalar.activation(
                    out=vss[:], in_=vss[:],
                    func=mybir.ActivationFunctionType.Sqrt, scale=float(DA),
                )
                st = s3.tile([nT, 1], F32)
                nc.vector.reciprocal(out=st[:], in_=vss[:])
                nc.vector.tensor_scalar_mul(
                    out=st[:], in0=st[:], scalar1=sca_s[:, 0:1]
                )
                # vt^T as bf16 for the similarity matmul
                vttb = s3.tile([128, 4, nT], BF16)
                for do in range(4):
                    vtp2 = ps3t.tile([128, nT], F32, tag="ptp")
                    nc.tensor.transpose(
                        vtp2[:], vt[:, 128 * do:128 * (do + 1)], idf[:nT, :nT]
                    )
                    nc.vector.tensor_copy(out=vttb[:, do, :], in_=vtp2[:])

                # ---- stage 4: similarity vs all audio tokens ----
                for lc in range(4):
                    smp = ps3.tile([nT, 512], F32, tag="pp")
                    for do in range(4):
                        nc.tensor.matmul(
                            smp[:],
                            vttb[:, do, :],
                            audtb[:, 4 * lc:4 * (lc + 1), do, :],
                            start=(do == 0),
                            stop=(do == 3),
                        )
                    o1 = s3.tile([nT, 512], F32, tag="o1")
                    nc.vector.tensor_scalar_mul(out=o1[:], in0=smp[:], scalar1=st[:])
                    nc.vector.tensor_tensor(
                        out=o1[:], in0=o1[:],
                        in1=rl_b[:nT, 512 * lc:512 * (lc + 1)],
                        op=mybir.AluOpType.mult,
                    )
                    nc.vector.tensor_scalar_add(
                        out=o1[:], in0=o1[:],
                        scalar1=sca_s[:, 1:2],
                    )
                    nc.scalar.dma_start(out=out[:, 512 * lc:512 * (lc + 1)], in_=o1[:])

    nc.compile()
    return nc


def host_fold(probe, wq, wk, bq, wv, bv, wo, bo, ln_g, ln_b, proj_w, proj_b,
              logit_scale, logit_bias):
    """Fold weights on the host into device-friendly layouts."""
    f64 = np.float64
    qvec = probe.reshape(-1).astype(f64) @ wq.astype(f64).T + bq.astype(f64)
    q = qvec.reshape(NH, DH)
    ck = np.stack(
        [q[h] @ wk.astype(f64)[h * DH:(h + 1) * DH, :] for h in range(NH)]
    )  # [16, 1024]
    ck /= np.sqrt(f64(DH))
    # cktb[ci, co*16+h] = ck[h, co*128+ci]
    ckt = ck.T.reshape(8, 128, NH).transpose(1, 0, 2).reshape(128, 128)
    cktb = ckt.astype(ml_dtypes.bfloat16)

    wvt = np.ascontiguousarray(
        wv.T.reshape(8, 128, DV).transpose(1, 0, 2)).astype(ml_dtypes.bfloat16)
    wot = np.ascontiguousarray(
        wo.T.reshape(8, 128, DV).transpose(1, 0, 2)).astype(ml_dtypes.bfloat16)
    pjt = np.ascontiguousarray(
        proj_w.T.reshape(8, 128, DA).transpose(1, 0, 2)).astype(ml_dtypes.bfloat16)
    bvp = np.ascontiguousarray(bv.reshape(8, 128).T).astype(np.float32)
    sca = np.array([[np.exp(np.float64(logit_scale[0])), logit_bias[0]]],
                   np.float32)
    return dict(
        cktb=np.ascontiguousarray(cktb),
        wvt=wvt, bvp=bvp, wot=wot,
        bo2=bo.reshape(1, DV).astype(np.float32),
        lng=ln_g.reshape(1, DV).astype(np.float32),
        lnb=ln_b.reshape(1, DV).astype(np.float32),
        pjt=pjt, pjb=proj_b.reshape(1, DA).astype(np.float32),
        sca=sca,
    )


_NC_CACHE = {}


def kernel(video_x, audio_x, probe, wq, wk, wv, bq, bk, bv, wo, bo,
           ln_g, ln_b, proj_w, proj_b, logit_scale, logit_bias, T=None, H=None,
           W=None, **_unused):
    from concourse.bass_utils import run_bass_kernel_spmd

    video_x = np.asarray(video_x, np.float32)
    audio_x = np.asarray(audio_x, np.float32)
    params = host_fold(
        np.asarray(probe, np.float32), np.asarray(wq, np.float32),
        np.asarray(wk, np.float32), np.asarray(bq, np.float32),
        np.asarray(wv, np.float32), np.asarray(bv, np.float32),
        np.asarray(wo, np.float32), np.asarray(bo, np.float32),
        np.asarray(ln_g, np.float32), np.asarray(ln_b, np.float32),
        np.asarray(proj_w, np.float32), np.asarray(proj_b, np.float32),
        np.asarray(logit_scale, np.float32), np.asarray(logit_bias, np.float32),
    )
    if "nc" not in _NC_CACHE:
        _NC_CACHE["nc"] = build_nc()
    nc = _NC_CACHE["nc"]
    in_maps = []
    for b in range(B):
        m = dict(params)
        m["xv"] = np.ascontiguousarray(video_x[b])
        m["aud"] = np.ascontiguousarray(audio_x[b])
        in_maps.append(m)
    res = run_bass_kernel_spmd(nc, in_maps, core_ids=list(range(B)), trace=False)
    return np.stack([res.results[b]["out"] for b in range(B)], axis=0)

